# Initial kernel scaffold
#
"""Bahdanau attention (B=32, S=2048, ENC2=1024, ATT=512) on 8 TRN2
NeuronCores, data-parallel over batch (4 batches/core), weights replicated.

Per-core program (Bass/Tile, fp32 in/out, bf16 main matmuls):
  Ws   = dec @ W_w + W_b + U_b              (prologue, PE, f32r)
  encB = bf16(enc)                          (Pool cast, one pass/block)
  encT = per-[128,128] PE transposes of encB (bf16: 1.0 cyc/row vs
         f32r's 1.5 -- saves ~0.85us/block of PE time)
  UhT  = bf16(U_w)^T-chunk @ encT           [a 128, s 512] psum tiles
  tanh = tanh(UhT + Ws) via ACT bias fusion (per-partition bias)
  en   = v^T tanh                           (PE)
  alpha= exp(en)/sum exp(en)                (ACT exp + fused row-sum)

The PE is software-pipelined one period (two s-blocks) deep: the matmul
chains for blocks (2p-2, 2p-1) are interleaved with the transposes of
blocks (2p, 2p+1), so the PSUM-evacuation copies always have a full
chain-length window to drain and the PE stays busy. The head is special-
cased (W_w lands right after enc block 0 so the Ws chain and block-0
chains run before T(1)); the tail flushes each energy row as soon as its
block's chains retire. An fp8-DoubleRow variant of the U@enc matmul was
tried and passes accuracy (3.2e-3 via e8+r8/U8a+U8b splitting), but on
real silicon DoubleRow matmuls run ~2.4x the cost model (ldweights+
matmult SEQ serialization), making it 60% slower than this version (bf16 non-DR
matmuls do NOT hit that penalty; measured at parity-or-better on HW).

Output alpha [32, 2048] fp32, gathered from the 8 cores.
"""

import numpy as np

import concourse.bass as bass
import concourse.mybir as mybir
import concourse.tile as tile
from concourse import bacc
from concourse.masks import make_identity

F32 = mybir.dt.float32
F32R = mybir.dt.float32r
F8 = mybir.dt.float8e4
BF16 = mybir.dt.bfloat16
DR = mybir.MatmulPerfMode.DoubleRow
COPY = mybir.ActivationFunctionType.Copy

N_CORES = 8
B_FULL, S, E, A = 32, 2048, 1024, 512
B_SH = B_FULL // N_CORES          # 4 batches per core
SBLK = 512                        # s-block (matmul N)
N_SBLK = S // SBLK                # 4 per batch
EJ = E // 128                     # 8 e-chunks
AM = A // 128                     # 4 a-chunks
CC = SBLK // 128                  # 4 s-subchunks per s-block
SU = 4096.0                       # fp8 scale for U (undone in tanh)


def r(ap):
    return ap.bitcast(F32R)


def build_program(reps=1):
    nc = bacc.Bacc("TRN2", target_bir_lowering=False, debug=False,
                   num_devices=N_CORES)

    dec = nc.dram_tensor("decoder_hidden", [B_SH, E], F32R, kind="ExternalInput")
    enc = nc.dram_tensor("encoder_all_hidden", [B_SH, S, E], F32R,
                         kind="ExternalInput")
    W_w = nc.dram_tensor("W_w", [E, A], F32R, kind="ExternalInput")
    W_b = nc.dram_tensor("W_b", [A], F32R, kind="ExternalInput")
    U_w = nc.dram_tensor("U_w", [E, A], F32R, kind="ExternalInput")
    U_b = nc.dram_tensor("U_b", [A], F32R, kind="ExternalInput")
    v_w = nc.dram_tensor("v_w", [A, 1], F32R, kind="ExternalInput")
    alpha = nc.dram_tensor("alpha", [B_SH, S], F32, kind="ExternalOutput")

    with tile.TileContext(nc) as tc:
        with (
            tc.tile_pool(name="const", bufs=1) as constp,
            tc.tile_pool(name="uws", bufs=3) as uwsp,
            tc.tile_pool(name="x4", bufs=3) as x4p,
            tc.tile_pool(name="x4b", bufs=3) as x4bp,
            tc.tile_pool(name="e8", bufs=3) as e8p,
            tc.tile_pool(name="scr", bufs=6) as scrp,
            tc.tile_pool(name="tanh", bufs=7) as tanhp,
            tc.tile_pool(name="epi", bufs=2) as epip,
            tc.tile_pool(name="psT", bufs=3, space="PSUM") as psTp,
            tc.tile_pool(name="psUh", bufs=3, space="PSUM") as psUhp,
            tc.tile_pool(name="psE", bufs=2, space="PSUM") as psEp,
        ):
            # ---------------- prologue: tiny inputs ---------------------
            ident_f32 = constp.tile([128, 128], F32, tag="identf")
            make_identity(nc, ident_f32)
            ident = constp.tile([128, 128], F32R, tag="ident")
            nc.vector.tensor_copy(ident, ident_f32)
            identb = constp.tile([128, 128], BF16, tag="identb")
            nc.vector.tensor_copy(identb, ident_f32)

            dec_sb = constp.tile([B_SH, E], F32R, tag="dec")
            nc.sync.dma_start(dec_sb, dec[:, :])
            wb_sb = constp.tile([1, A], F32R, tag="wb")
            nc.sync.dma_start(wb_sb, W_b[None, :])
            ub_sb = constp.tile([1, A], F32R, tag="ub")
            nc.sync.dma_start(ub_sb, U_b[None, :])
            v_sb = constp.tile([128, AM], F32R, tag="v")
            nc.sync.dma_start(v_sb.rearrange("p (c o) -> p c o", c=AM),
                              v_w.rearrange("(c p) o -> p c o", p=128))

            # dec transposes first: PE work that's ready immediately
            dect = constp.tile([128, B_SH * EJ], F32R, tag="dect")
            for j in range(EJ):
                pst = psTp.tile([128, 128], F32R, tag="psT")
                nc.tensor.transpose(r(pst[:, :B_SH]),
                                    r(dec_sb[:, 128 * j:128 * (j + 1)]),
                                    r(ident[:B_SH, :B_SH]))
                nc.scalar.copy(dect[:, B_SH * j:B_SH * (j + 1)], pst[:, :B_SH])

            # first enc block DMA before the weight loads, split per
            # 128-row chunk so transposes can start on chunk 0
            x4_first = x4p.tile([128, CC * E], F32R, tag="x4")
            for c in range(CC):
                nc.sync.dma_start(x4_first[:, E * c:E * (c + 1)],
                                  enc[0, 128 * c:128 * (c + 1), :])

            # W_w right after enc block 0 (the Ws chain gates the first
            # tanh); then U_w split per chunk so the first MM chain can
            # consume chunks as they land
            ww = constp.tile([128, EJ * A], F32R, tag="ww")
            nc.sync.dma_start(ww.rearrange("e (j a) -> e j a", j=EJ),
                              W_w.rearrange("(j e) a -> e j a", e=128))
            uw = constp.tile([128, EJ * A], F32R, tag="uw")
            uwb = constp.tile([128, EJ * A], BF16, tag="uwb")
            for j in range(EJ):
                sl = slice(A * j, A * (j + 1))
                nc.sync.dma_start(uw[:, sl], U_w[128 * j:128 * (j + 1), :])
                e0 = nc.gpsimd if j % 2 == 0 else nc.vector
                e0.tensor_copy(uwb[:, sl], uw.bitcast(F32)[:, sl])

            x4_second = x4p.tile([128, CC * E], F32R, tag="x4",
                                 name="x4_1")
            nc.sync.dma_start(
                x4_second.rearrange("p (c e) -> p c e", c=CC),
                enc[0, SBLK:2 * SBLK, :].rearrange("(c p) e -> p c e", p=128))

            bias_sum = constp.tile([1, A], F32R, tag="bias")
            nc.vector.tensor_tensor(out=bias_sum, in0=wb_sb, in1=ub_sb,
                                    op=mybir.AluOpType.add)
            ones14f = constp.tile([1, B_SH], F32, tag="onesf")
            nc.vector.memset(ones14f, 1.0)
            ones14 = constp.tile([1, B_SH], F32R, tag="ones")
            nc.vector.tensor_copy(ones14, ones14f)
            onescf = constp.tile([128, 1], F32, tag="onescf")
            nc.vector.memset(onescf, 1.0)
            onesc = constp.tile([128, 1], F32R, tag="onesc")
            nc.vector.tensor_copy(onesc, onescf)


            wst = constp.tile([128, AM * B_SH], F32, tag="wst")

            def prologue_part2():
                # Ws = dec @ W_w + (W_b + U_b):  psum [B_SH, A]
                ps_ws = psEp.tile([B_SH, A], F32, tag="psE", name="ps_ws")
                for j in range(EJ):
                    nc.tensor.matmul(ps_ws,
                                     r(dect[:, B_SH * j:B_SH * (j + 1)]),
                                     r(ww[:, A * j:A * (j + 1)]),
                                     start=(j == 0), stop=False)
                nc.tensor.matmul(ps_ws, r(ones14), r(bias_sum),
                                 start=False, stop=True)
                ws_sb = constp.tile([B_SH, A], F32R, tag="ws", name="ws_sb")
                nc.scalar.copy(ws_sb, ps_ws)
                # WsT [128 a', (m b)]: col 4m+b = Ws[b, 128m + p]
                for m in range(AM):
                    pst = psTp.tile([128, 128], F32R, tag="psT",
                                    name=f"pst_ws_{m}")
                    nc.tensor.transpose(r(pst[:, :B_SH]),
                                        r(ws_sb[:, 128 * m:128 * (m + 1)]),
                                        r(ident[:B_SH, :B_SH]))
                    nc.scalar.copy(wst[:, B_SH * m:B_SH * (m + 1)],
                                   pst[:, :B_SH])

            # Per-j engine assignment for the evacuation passes, aligned
            # to DoubleRow j-pairs so each MM instr waits on few producer
            # engines. Constraint: GPSIMD (Pool) cannot touch PSUM on HW,
            # so only DVE/ACT read pst; Pool serves j6/j7 from an f32
            # SBUF scratch that ACT evacuates (plus the epilogue mul).
            CAST = [nc.scalar, nc.scalar, nc.scalar, nc.scalar,
                    nc.vector, nc.vector, nc.gpsimd, nc.gpsimd]
            SUB = [nc.vector, nc.vector, nc.vector, nc.vector,
                   nc.vector, nc.vector, nc.gpsimd, nc.gpsimd]
            VIA_SCR = (6, 7)

            # ---------------- software-pipelined main loop --------------
            blocks = [(rep, b, sblk)
                      for rep in range(reps)
                      for b in range(B_SH)
                      for sblk in range(N_SBLK)]
            n = len(blocks)

            x4_tiles = {0: x4_first, 1: x4_second}
            e8_tiles = {}
            r8_tiles = {}
            th_tiles = {}
            stot_tiles = {}
            batch_state = {}   # (rep, b) -> (exp_b, den_b)
            pending = []       # block indices with tanh done, energy not

            def get_batch_state(rep, b):
                key = (rep, b)
                if key not in batch_state:
                    exp_b = epip.tile([1, S], F32, tag="exp",
                                      name=f"exp_{rep}_{b}")
                    den_b = epip.tile([1, N_SBLK], F32, tag="den",
                                      name=f"den_{rep}_{b}")
                    batch_state[key] = (exp_b, den_b)
                return batch_state[key]

            def flush_energy():
                g = pending.pop(0)
                rep, b, sblk = blocks[g]
                exp_b, den_b = get_batch_state(rep, b)
                s0 = SBLK * sblk
                ths = th_tiles.pop(g)
                if g < 2 or g >= n - 2:
                    # head/tail blocks: PE is idle there, so the direct
                    # 4-matmul reduction has the shortest latency
                    ps_e = psEp.tile([1, SBLK], F32, tag="psE",
                                     name=f"psE_{rep}_{b}_{sblk}")
                    for m in range(AM):
                        nc.tensor.matmul(ps_e, r(v_sb[:, m:m + 1]),
                                         r(ths[m]), start=(m == 0),
                                         stop=(m == AM - 1))
                    nc.scalar.activation(
                        out=exp_b[:, s0:s0 + SBLK], in_=ps_e,
                        func=mybir.ActivationFunctionType.Exp,
                        accum_out=den_b[:, sblk:sblk + 1])
                    if sblk == N_SBLK - 1:
                        finish_batch(rep, b, exp_b, den_b)
                    return
                # v (x) tanh on the spare ACT/DVE/Pool capacity, so the PE
                # does a single ones-column matmul (512 cyc) per energy row
                # instead of four v-column matmuls (2048 cyc)
                vth = []
                for m in range(AM):
                    w = scrp.tile([128, SBLK], F32R, tag="vth",
                                  name=f"vth_{g}_{m}")
                    nc.scalar.activation(out=w, in_=ths[m].bitcast(F32),
                                         func=COPY,
                                         scale=v_sb.bitcast(F32)[:, m:m + 1])
                    vth.append(w)
                s01 = scrp.tile([128, SBLK], F32R, tag="vth",
                                name=f"vth01_{g}")
                nc.vector.tensor_tensor(out=s01, in0=vth[0], in1=vth[1],
                                        op=mybir.AluOpType.add)
                s23 = scrp.tile([128, SBLK], F32R, tag="vth",
                                name=f"vth23_{g}")
                nc.gpsimd.tensor_tensor(out=s23, in0=vth[2], in1=vth[3],
                                        op=mybir.AluOpType.add)
                stot = scrp.tile([128, SBLK], F32R, tag="vth",
                                 name=f"vthT_{g}")
                nc.vector.tensor_tensor(out=stot, in0=s01, in1=s23,
                                        op=mybir.AluOpType.add)
                ps_e = psEp.tile([1, SBLK], F32, tag="psE",
                                 name=f"psE_{rep}_{b}_{sblk}")
                nc.tensor.matmul(ps_e, onesc, r(stot), start=True, stop=True)
                nc.scalar.activation(out=exp_b[:, s0:s0 + SBLK],
                                     in_=ps_e,
                                     func=mybir.ActivationFunctionType.Exp,
                                     accum_out=den_b[:, sblk:sblk + 1])
                if sblk == N_SBLK - 1:
                    finish_batch(rep, b, exp_b, den_b)

            def finish_batch(rep, b, exp_b, den_b):
                if True:
                    # batch b complete: softmax epilogue
                    # (no max subtraction; |energy| <= 22.6)
                    dsum_b = epip.tile([1, 1], F32, tag="dsum",
                                       name=f"dsum_{rep}_{b}")
                    nc.vector.reduce_sum(dsum_b, den_b,
                                         axis=mybir.AxisListType.X)
                    inv_b = epip.tile([1, 1], F32, tag="inv",
                                      name=f"inv_{rep}_{b}")
                    nc.vector.reciprocal(inv_b, dsum_b)
                    h = S // 2
                    nc.gpsimd.tensor_scalar_mul(exp_b[:, :h], exp_b[:, :h],
                                                inv_b)
                    nc.vector.tensor_scalar_mul(exp_b[:, h:], exp_b[:, h:],
                                                inv_b)
                    nc.sync.dma_start(alpha[b:b + 1, :h], exp_b[:, :h])
                    nc.scalar.dma_start(alpha[b:b + 1, h:], exp_b[:, h:])
                    del batch_state[(rep, b)]

            def issue_dma(g):
                repn, bn, sblkn = blocks[g]
                s0n = SBLK * sblkn
                x4n = x4p.tile([128, CC * E], F32R, tag="x4",
                               name=f"x4_{g}")
                nc.sync.dma_start(
                    x4n.rearrange("p (c e) -> p c e", c=CC),
                    enc[bn, s0n:s0n + SBLK, :]
                    .rearrange("(c p) e -> p c e", p=128))
                x4_tiles[g] = x4n

            x4b_tiles = {}

            def cast_block(g, chunked=False):
                # f32 -> bf16 on the (otherwise idle) Pool engine so the
                # PE transposes run at 1.0 cyc/row instead of f32r's 1.5;
                # bf16 enc costs ~3.6e-3 max-rel end to end (within budget)
                x4 = x4_tiles.pop(g)
                x4b = x4bp.tile([128, CC * E], BF16, tag="x4b",
                                name=f"x4b_{g}")
                if chunked:
                    for c in range(CC):
                        nc.gpsimd.tensor_copy(x4b[:, E * c:E * (c + 1)],
                                              x4.bitcast(F32)
                                              [:, E * c:E * (c + 1)])
                else:
                    nc.gpsimd.tensor_copy(x4b, x4.bitcast(F32))
                x4b_tiles[g] = x4b

            def transpose_evac(g, js):
                if g not in e8_tiles:
                    e8 = e8p.tile([128, EJ * SBLK], BF16, tag="e8",
                                  name=f"e8_{g}")
                    e8_tiles[g] = e8
                else:
                    e8 = e8_tiles[g]
                x4b = x4b_tiles[g]
                if g == 0:
                    # block 0 only: c-major transposes with narrow evacs so
                    # PE starts on the first 128-row DMA chunk immediately
                    for c in range(CC):
                        for j in js:
                            pstn = psTp.tile([128, 128], BF16, tag="psT",
                                             name=f"pstn_{c}_{j}")
                            nc.tensor.transpose(
                                pstn,
                                x4b[:, E * c + 128 * j:
                                    E * c + 128 * (j + 1)],
                                identb)
                            dst = e8[:, SBLK * j + 128 * c:
                                     SBLK * j + 128 * (c + 1)]
                            if (c * len(js) + j) % 5 == 4:
                                nc.scalar.copy(dst, pstn)
                            else:
                                nc.vector.tensor_copy(dst, pstn)
                    return
                for j in js:
                    pst = psTp.tile([128, SBLK], BF16, tag="psT")
                    for c in range(CC):
                        nc.tensor.transpose(
                            pst[:, 128 * c:128 * (c + 1)],
                            x4b[:, E * c + 128 * j:E * c + 128 * (j + 1)],
                            identb)
                    sl = slice(SBLK * j, SBLK * (j + 1))
                    if j in (2, 5):
                        nc.scalar.copy(e8[:, sl], pst)
                    else:
                        nc.vector.tensor_copy(e8[:, sl], pst)

            def mm_chain(g, m, ths):
                # one m-chain for block g: 12 DoubleRow MMs (K=256 each)
                # accumulating e8@U8a + e8@U8b + r8@U8a into one psum tile
                rep, b, sblk = blocks[g]
                e8 = e8_tiles[g]
                ps_uh = psUhp.tile([128, SBLK], F32, tag="psUh")
                for j in range(EJ):
                    nc.tensor.matmul(
                        ps_uh,
                        uwb[:, A * j + 128 * m:A * j + 128 * (m + 1)],
                        e8[:, SBLK * j:SBLK * (j + 1)],
                        start=(j == 0), stop=(j == EJ - 1))
                th = tanhp.tile([128, SBLK], F32R, tag="tanh",
                                name=f"tanh_{g}_{m}")
                nc.scalar.activation(
                    out=th, in_=ps_uh,
                    func=mybir.ActivationFunctionType.Tanh,
                    bias=wst[:, B_SH * m + b:B_SH * m + b + 1])
                ths.append(th)
                if m == AM - 1:
                    th_tiles[g] = ths
                    pending.append(g)
                    del e8_tiles[g]

            # Period p: transposes+evacs for blocks (2p, 2p+1) interleaved
            # with the MM chains for blocks (2p-2, 2p-1) and energy for
            # blocks (2p-4, 2p-3). Emitting transposes in 4-j groups
            # between chains gives the psT pool (3 banks) a chain-length
            # window to drain, and MM inputs are a full period old.
            n_super = (n + 1) // 2
            for p in range(n_super + 2):
                g0, g1 = 2 * p, 2 * p + 1
                have_T = g0 < n
                gA, gB = g0 - 2, g1 - 2
                if have_T:
                    if g1 + 1 < n and (g1 + 1) not in x4_tiles:
                        issue_dma(g1 + 1)
                    if g1 + 2 < n and (g1 + 2) not in x4_tiles:
                        issue_dma(g1 + 2)
                thA, thB = [], []
                if p == 0:
                    # head: T(0) -> Ws chain -> block-0 chains -> T(1).
                    # The Ws/wst writes are emitted before any tanh, which
                    # is what keeps the cold-device read-after-write on
                    # wst correct (same-engine program order).
                    cast_block(0, chunked=True)
                    transpose_evac(0, range(0, EJ))
                    prologue_part2()
                    th0 = []
                    for m in range(AM):
                        mm_chain(0, m, th0)
                    cast_block(1)
                    transpose_evac(1, range(0, EJ))
                    if 2 < n:
                        cast_block(2)
                    continue
                if have_T and g1 in x4_tiles:
                    cast_block(g1)
                if gA == 0:
                    gA = -1   # block 0 already processed in the head
                if 0 <= gA < n:
                    mm_chain(gA, 0, thA)
                    mm_chain(gA, 1, thA)
                if have_T:
                    transpose_evac(g0, range(0, 4))
                if 0 <= gA < n:
                    mm_chain(gA, 2, thA)
                    mm_chain(gA, 3, thA)
                if have_T:
                    transpose_evac(g0, range(4, EJ))
                if not have_T and pending:
                    flush_energy()
                if p >= 2 and pending:
                    flush_energy()
                if 0 <= gB < n:
                    mm_chain(gB, 0, thB)
                    mm_chain(gB, 1, thB)
                if have_T:
                    transpose_evac(g1, range(0, 4))
                if 0 <= gB < n:
                    mm_chain(gB, 2, thB)
                    mm_chain(gB, 3, thB)
                if have_T:
                    transpose_evac(g1, range(4, EJ))
                if have_T and (g1 + 1) in x4_tiles and g1 + 1 < n:
                    cast_block(g1 + 1)
                if not have_T and pending:
                    flush_energy()
                if p >= 2 and pending and (len(pending) > 2 or not have_T):
                    flush_energy()
            while pending:
                flush_energy()

    nc.compile()
    return nc


def shard_inputs(inputs):
    """Full inputs dict -> list of 8 per-core input dicts."""
    dec = np.ascontiguousarray(inputs["decoder_hidden"], dtype=np.float32)
    enc = np.ascontiguousarray(inputs["encoder_all_hidden"], dtype=np.float32)
    base = {
        "W_w": np.ascontiguousarray(inputs["W_w"], dtype=np.float32),
        "W_b": np.ascontiguousarray(inputs["W_b"], dtype=np.float32),
        "U_w": np.ascontiguousarray(inputs["U_w"], dtype=np.float32),
        "U_b": np.ascontiguousarray(inputs["U_b"], dtype=np.float32),
        "v_w": np.ascontiguousarray(inputs["v_w"], dtype=np.float32),
    }
    maps = []
    for c in range(N_CORES):
        m = dict(base)
        m["decoder_hidden"] = dec[c * B_SH:(c + 1) * B_SH]
        m["encoder_all_hidden"] = enc[c * B_SH:(c + 1) * B_SH]
        maps.append(m)
    return maps


_NC_CACHE = None


def get_program():
    global _NC_CACHE
    if _NC_CACHE is None:
        _NC_CACHE = build_program()
    return _NC_CACHE


def kernel(**inputs):
    from concourse import bass_utils
    nc = get_program()
    maps = shard_inputs(inputs)
    res = bass_utils.run_bass_kernel_spmd(nc, maps,
                                          core_ids=list(range(N_CORES)))
    return np.concatenate([res.results[c]["alpha"] for c in range(N_CORES)],
                          axis=0)



# revision 2
# speedup vs baseline: 1.0375x; 1.0375x over previous
"""Bahdanau attention (B=32, S=2048, ENC2=1024, ATT=512) on 8 TRN2
NeuronCores, data-parallel over batch (4 batches/core), weights replicated.

v3: enc loads as f32 (plain HWDGE DMA), is cast f32->bf16 on Pool+DVE,
and the [s,e]->[e,s] transposes run on the DMA xbar (dma_start_transpose,
SBUF->SBUF bf16) instead of the PE. (SWDGE cast-DMA would halve the cast
work but corrupts concurrently-running xbar transposes -- shared S2M
quantize unit -- so loads stay on HWDGE.) The PE does only the real
matmuls:
  Ws   = dec @ W_w + W_b + U_b                  (prologue)
  UhT  = bf16(U)^T-chunk @ encT                 [a 128, s 512] psum tiles
  tanh = tanh(UhT + Ws) via ACT bias fusion     (per-partition bias)
  en   = ones^T (v (x) tanh)  -- v-scaling on ACT, adds on DVE/Pool,
         single ones-column matmul per s-block on PE
  alpha= exp(en)/sum exp(en)  (ACT exp + fused row-sum, DVE/Pool scale)

Output alpha [32, 2048] fp32, gathered from the 8 cores.
"""

import numpy as np

import concourse.bass as bass
import concourse.mybir as mybir
import concourse.tile as tile
from concourse import bacc
from concourse.masks import make_identity
from concourse.tile_rust import add_dep_helper

F32 = mybir.dt.float32
F32R = mybir.dt.float32r
BF16 = mybir.dt.bfloat16
COPY = mybir.ActivationFunctionType.Copy

N_CORES = 8
B_FULL, S, E, A = 32, 2048, 1024, 512
B_SH = B_FULL // N_CORES          # 4 batches per core
SBLK = 512                        # s-block (matmul N)
N_SBLK = S // SBLK                # 4 per batch
EJ = E // 128                     # 8 e-chunks
AM = A // 128                     # 4 a-chunks
CC = SBLK // 128                  # 4 s-subchunks per s-block


def r(ap):
    return ap.bitcast(F32R)


def build_program(reps=1):
    nc = bacc.Bacc("TRN2", target_bir_lowering=False, debug=False,
                   num_devices=N_CORES)

    dec = nc.dram_tensor("decoder_hidden", [B_SH, E], F32R, kind="ExternalInput")
    enc = nc.dram_tensor("encoder_all_hidden", [B_SH, S, E], F32,
                         kind="ExternalInput")
    W_w = nc.dram_tensor("W_w", [E, A], F32R, kind="ExternalInput")
    W_b = nc.dram_tensor("W_b", [A], F32R, kind="ExternalInput")
    U_w = nc.dram_tensor("U_w", [E, A], F32R, kind="ExternalInput")
    U_b = nc.dram_tensor("U_b", [A], F32R, kind="ExternalInput")
    v_w = nc.dram_tensor("v_w", [A, 1], F32R, kind="ExternalInput")
    alpha = nc.dram_tensor("alpha", [B_SH, S], F32, kind="ExternalOutput")

    with tile.TileContext(nc) as tc:
        with (
            tc.tile_pool(name="const", bufs=1) as constp,
            tc.tile_pool(name="x4", bufs=3) as x4p,
            tc.tile_pool(name="x4b", bufs=3) as x4bp,
            tc.tile_pool(name="e8", bufs=3) as e8p,
            tc.tile_pool(name="scr", bufs=6) as scrp,
            tc.tile_pool(name="tanh", bufs=7) as tanhp,
            tc.tile_pool(name="epi", bufs=2) as epip,
            tc.tile_pool(name="psT", bufs=2, space="PSUM") as psTp,
            tc.tile_pool(name="psUh", bufs=3, space="PSUM") as psUhp,
            tc.tile_pool(name="psE", bufs=2, space="PSUM") as psEp,
        ):
            # ---------------- prologue: tiny inputs ---------------------
            ident_f32 = constp.tile([128, 128], F32, tag="identf")
            make_identity(nc, ident_f32)
            ident = constp.tile([128, 128], F32R, tag="ident")
            nc.vector.tensor_copy(ident, ident_f32)

            dec_sb = constp.tile([B_SH, E], F32R, tag="dec")
            nc.sync.dma_start(dec_sb, dec[:, :])
            wb_sb = constp.tile([1, A], F32R, tag="wb")
            nc.sync.dma_start(wb_sb, W_b[None, :])
            ub_sb = constp.tile([1, A], F32R, tag="ub")
            nc.sync.dma_start(ub_sb, U_b[None, :])
            v_sb = constp.tile([128, AM], F32R, tag="v")
            nc.sync.dma_start(v_sb.rearrange("p (c o) -> p c o", c=AM),
                              v_w.rearrange("(c p) o -> p c o", p=128))

            # dec transposes first: PE work that's ready immediately
            dect = constp.tile([128, B_SH * EJ], F32R, tag="dect")
            for j in range(EJ):
                pst = psTp.tile([128, 128], F32R, tag="psT")
                nc.tensor.transpose(r(pst[:, :B_SH]),
                                    r(dec_sb[:, 128 * j:128 * (j + 1)]),
                                    r(ident[:B_SH, :B_SH]))
                nc.scalar.copy(dect[:, B_SH * j:B_SH * (j + 1)], pst[:, :B_SH])

            # weights
            ww = constp.tile([128, EJ * A], F32R, tag="ww")
            nc.sync.dma_start(ww.rearrange("e (j a) -> e j a", j=EJ),
                              W_w.rearrange("(j e) a -> e j a", e=128))
            uw = constp.tile([128, EJ * A], F32R, tag="uw")
            uwb = constp.tile([128, EJ * A], BF16, tag="uwb")
            for j in range(EJ):
                sl = slice(A * j, A * (j + 1))
                nc.sync.dma_start(uw[:, sl], U_w[128 * j:128 * (j + 1), :])
                e0 = nc.gpsimd if j % 2 == 0 else nc.vector
                e0.tensor_copy(uwb[:, sl], uw.bitcast(F32)[:, sl])

            bias_sum = constp.tile([1, A], F32R, tag="bias")
            nc.vector.tensor_tensor(out=bias_sum, in0=wb_sb, in1=ub_sb,
                                    op=mybir.AluOpType.add)
            ones14f = constp.tile([1, B_SH], F32, tag="onesf")
            nc.vector.memset(ones14f, 1.0)
            ones14 = constp.tile([1, B_SH], F32R, tag="ones")
            nc.vector.tensor_copy(ones14, ones14f)
            onescf = constp.tile([128, 1], F32, tag="onescf")
            nc.vector.memset(onescf, 1.0)
            onesc = constp.tile([128, 1], F32R, tag="onesc")
            nc.vector.tensor_copy(onesc, onescf)

            wst = constp.tile([128, AM * B_SH], F32, tag="wst")

            def prologue_part2():
                # Ws = dec @ W_w + (W_b + U_b):  psum [B_SH, A]
                ps_ws = psEp.tile([B_SH, A], F32, tag="psE", name="ps_ws")
                for j in range(EJ):
                    nc.tensor.matmul(ps_ws,
                                     r(dect[:, B_SH * j:B_SH * (j + 1)]),
                                     r(ww[:, A * j:A * (j + 1)]),
                                     start=(j == 0), stop=False)
                nc.tensor.matmul(ps_ws, r(ones14), r(bias_sum),
                                 start=False, stop=True)
                ws_sb = constp.tile([B_SH, A], F32R, tag="ws", name="ws_sb")
                nc.scalar.copy(ws_sb, ps_ws)
                # WsT [128 a', (m b)]: col 4m+b = Ws[b, 128m + p]
                for m in range(AM):
                    pst = psTp.tile([128, 128], F32R, tag="psT",
                                    name=f"pst_ws_{m}")
                    nc.tensor.transpose(r(pst[:, :B_SH]),
                                        r(ws_sb[:, 128 * m:128 * (m + 1)]),
                                        r(ident[:B_SH, :B_SH]))
                    nc.scalar.copy(wst[:, B_SH * m:B_SH * (m + 1)],
                                   pst[:, :B_SH])

            # ---------------- main loop ---------------------------------
            blocks = [(rep, b, sblk)
                      for rep in range(reps)
                      for b in range(B_SH)
                      for sblk in range(N_SBLK)]
            n = len(blocks)

            x4b_tiles = {}
            e8_tiles = {}
            th_tiles = {}
            batch_state = {}   # (rep, b) -> (exp_b, den_b)
            pending = []       # block indices with tanh done, energy not

            def get_batch_state(rep, b):
                key = (rep, b)
                if key not in batch_state:
                    exp_b = epip.tile([1, S], F32, tag="exp",
                                      name=f"exp_{rep}_{b}")
                    den_b = epip.tile([1, N_SBLK], F32, tag="den",
                                      name=f"den_{rep}_{b}")
                    batch_state[key] = (exp_b, den_b)
                return batch_state[key]

            x4_tiles = {}
            load_insts = {}
            lastT_insts = {}

            def issue_load(g, after=None):
                rep, b, sblk = blocks[g]
                s0 = SBLK * sblk
                x4 = x4p.tile([128, CC * E], F32, tag="x4",
                              name=f"x4_{g}")
                # plain HWDGE f32 load (SWDGE cast-DMA corrupts concurrent
                # xbar transposes -- shared S2M quantize unit)
                bi = nc.sync.dma_start(
                    x4.rearrange("p (c e) -> p c e", c=CC),
                    enc[b, s0:s0 + SBLK, :]
                    .rearrange("(c p) e -> p c e", p=128))
                if after is not None:
                    # loads and xbar transposes must not run concurrently
                    # (2.3x mutual slowdown on the SDMA/S2M path)
                    add_dep_helper(bi.ins, after,
                                   reason="serialize DMA track: load after T")
                x4_tiles[g] = x4
                load_insts[g] = bi.ins

            def issue_cast(g):
                # f32 -> bf16 on Pool (c 0-1) and DVE (c 2-3)
                x4 = x4_tiles.pop(g)
                x4b = x4bp.tile([128, CC * E], BF16, tag="x4b",
                                name=f"x4b_{g}")
                h = CC * E // 2
                nc.gpsimd.tensor_copy(x4b[:, :h], x4.bitcast(F32)[:, :h])
                nc.vector.tensor_copy(x4b[:, h:], x4.bitcast(F32)[:, h:])
                x4b_tiles[g] = x4b

            def issue_transposes(g, after=None):
                # 4 xbar transposes: x4b[:, c, :] ([s=128, e=1024]) ->
                # e8[:, c-slab] with layout (c j s): each transpose's
                # destination is per-partition CONTIGUOUS (the xbar writes
                # wrong data to strided destinations on HW). Chain j's
                # moving operand is then the 3D AP [p][c][s].
                x4b = x4b_tiles.pop(g)
                e8 = e8p.tile([128, CC * EJ * 128], BF16, tag="e8",
                              name=f"e8_{g}")
                e8v = e8.rearrange("p (c j s) -> p c j s", c=CC, j=EJ)
                for c in range(CC):
                    bi = nc.sync.dma_start_transpose(
                        e8v[:, c], x4b[:, E * c:E * (c + 1)])
                    if c == 0 and after is not None:
                        add_dep_helper(bi.ins, after,
                                       reason="serialize DMA track: T after load")
                lastT_insts[g] = bi.ins
                e8_tiles[g] = e8

            def flush_energy():
                g = pending.pop(0)
                rep, b, sblk = blocks[g]
                exp_b, den_b = get_batch_state(rep, b)
                s0 = SBLK * sblk
                ths = th_tiles.pop(g)
                if g < 1 or g >= n - 1:
                    # head/tail blocks: PE idle there; direct 4-matmul
                    # reduction has the shortest latency
                    ps_e = psEp.tile([1, SBLK], F32, tag="psE",
                                     name=f"psE_{rep}_{b}_{sblk}")
                    for m in range(AM):
                        nc.tensor.matmul(ps_e, r(v_sb[:, m:m + 1]),
                                         r(ths[m]), start=(m == 0),
                                         stop=(m == AM - 1))
                    nc.scalar.activation(
                        out=exp_b[:, s0:s0 + SBLK], in_=ps_e,
                        func=mybir.ActivationFunctionType.Exp,
                        accum_out=den_b[:, sblk:sblk + 1])
                    if sblk == N_SBLK - 1:
                        finish_batch(rep, b, exp_b, den_b)
                    return
                # v (x) tanh on DVE/Pool (fused mul+add via
                # scalar_tensor_tensor); PE does a single ones-column
                # matmul (512 cyc) per energy row; ACT keeps only tanh+exp
                a01 = scrp.tile([128, SBLK], F32R, tag="vth",
                                name=f"vth0_{g}")
                nc.vector.tensor_scalar_mul(a01, ths[0], v_sb.bitcast(F32)[:, 0:1])
                b01 = scrp.tile([128, SBLK], F32R, tag="vth",
                                name=f"vth01_{g}")
                nc.vector.scalar_tensor_tensor(
                    out=b01, in0=ths[1], scalar=v_sb.bitcast(F32)[:, 1:2], in1=a01,
                    op0=mybir.AluOpType.mult, op1=mybir.AluOpType.add)
                a23 = scrp.tile([128, SBLK], F32R, tag="vth",
                                name=f"vth2_{g}")
                nc.vector.tensor_scalar_mul(a23, ths[2], v_sb.bitcast(F32)[:, 2:3])
                b23 = scrp.tile([128, SBLK], F32R, tag="vth",
                                name=f"vth23_{g}")
                nc.vector.scalar_tensor_tensor(
                    out=b23, in0=ths[3], scalar=v_sb.bitcast(F32)[:, 3:4], in1=a23,
                    op0=mybir.AluOpType.mult, op1=mybir.AluOpType.add)
                stot = scrp.tile([128, SBLK], F32R, tag="vth",
                                 name=f"vthT_{g}")
                nc.gpsimd.tensor_tensor(out=stot, in0=b01, in1=b23,
                                        op=mybir.AluOpType.add)
                ps_e = psEp.tile([1, SBLK], F32, tag="psE",
                                 name=f"psE_{rep}_{b}_{sblk}")
                nc.tensor.matmul(ps_e, onesc, r(stot), start=True, stop=True)
                nc.scalar.activation(out=exp_b[:, s0:s0 + SBLK],
                                     in_=ps_e,
                                     func=mybir.ActivationFunctionType.Exp,
                                     accum_out=den_b[:, sblk:sblk + 1])
                if sblk == N_SBLK - 1:
                    finish_batch(rep, b, exp_b, den_b)

            def finish_batch(rep, b, exp_b, den_b):
                # softmax epilogue (no max subtraction; |energy| <= 22.6)
                dsum_b = epip.tile([1, 1], F32, tag="dsum",
                                   name=f"dsum_{rep}_{b}")
                nc.vector.reduce_sum(dsum_b, den_b,
                                     axis=mybir.AxisListType.X)
                inv_b = epip.tile([1, 1], F32, tag="inv",
                                  name=f"inv_{rep}_{b}")
                nc.vector.reciprocal(inv_b, dsum_b)
                h = S // 2
                nc.gpsimd.tensor_scalar_mul(exp_b[:, :h], exp_b[:, :h],
                                            inv_b)
                nc.vector.tensor_scalar_mul(exp_b[:, h:], exp_b[:, h:],
                                            inv_b)
                nc.sync.dma_start(alpha[b:b + 1, :h], exp_b[:, :h])
                nc.scalar.dma_start(alpha[b:b + 1, h:], exp_b[:, h:])
                del batch_state[(rep, b)]

            def mm_chain(g, m, ths):
                rep, b, sblk = blocks[g]
                e8 = e8_tiles[g]
                e8j = e8.rearrange("p (c j s) -> p j c s", c=CC, j=EJ)
                ps_uh = psUhp.tile([128, SBLK], F32, tag="psUh")
                for j in range(EJ):
                    nc.tensor.matmul(
                        ps_uh,
                        uwb[:, A * j + 128 * m:A * j + 128 * (m + 1)],
                        e8j[:, j],
                        start=(j == 0), stop=(j == EJ - 1))
                th = tanhp.tile([128, SBLK], F32R, tag="tanh",
                                name=f"tanh_{g}_{m}")
                nc.scalar.activation(
                    out=th, in_=ps_uh,
                    func=mybir.ActivationFunctionType.Tanh,
                    bias=wst[:, B_SH * m + b:B_SH * m + b + 1])
                ths.append(th)
                if m == AM - 1:
                    th_tiles[g] = ths
                    pending.append(g)
                    del e8_tiles[g]

            # software pipeline: loads 3 ahead, cast+transposes 1 ahead.
            # DMA track is a strict alternation ... T(g) -> L(g+3) ->
            # T(g+1) -> L(g+4) ... so a load transfer and an xbar
            # transpose never run concurrently; the engine casts hide in
            # the load windows.
            issue_load(0)
            issue_load(1)
            issue_load(2)
            issue_cast(0)
            prologue_part2()
            issue_transposes(0, after=load_insts[2])
            for g in range(n):
                if g + 3 < n:
                    issue_load(g + 3, after=lastT_insts[g])
                if g + 1 < n:
                    issue_cast(g + 1)
                    issue_transposes(g + 1,
                                     after=load_insts.get(g + 3))
                ths = []
                for m in range(AM):
                    mm_chain(g, m, ths)
                # flush the previous block's energy while this block's
                # chains run
                if pending and (g >= 2 or len(pending) > 1):
                    flush_energy()
            while pending:
                flush_energy()

    nc.compile()
    return nc


def shard_inputs(inputs):
    """Full inputs dict -> list of 8 per-core input dicts."""
    dec = np.ascontiguousarray(inputs["decoder_hidden"], dtype=np.float32)
    enc = np.ascontiguousarray(inputs["encoder_all_hidden"], dtype=np.float32)
    base = {
        "W_w": np.ascontiguousarray(inputs["W_w"], dtype=np.float32),
        "W_b": np.ascontiguousarray(inputs["W_b"], dtype=np.float32),
        "U_w": np.ascontiguousarray(inputs["U_w"], dtype=np.float32),
        "U_b": np.ascontiguousarray(inputs["U_b"], dtype=np.float32),
        "v_w": np.ascontiguousarray(inputs["v_w"], dtype=np.float32),
    }
    maps = []
    for c in range(N_CORES):
        m = dict(base)
        m["decoder_hidden"] = dec[c * B_SH:(c + 1) * B_SH]
        m["encoder_all_hidden"] = enc[c * B_SH:(c + 1) * B_SH]
        maps.append(m)
    return maps


_NC_CACHE = None


def get_program():
    global _NC_CACHE
    if _NC_CACHE is None:
        _NC_CACHE = build_program()
    return _NC_CACHE


def kernel(**inputs):
    from concourse import bass_utils
    nc = get_program()
    maps = shard_inputs(inputs)
    res = bass_utils.run_bass_kernel_spmd(nc, maps,
                                          core_ids=list(range(N_CORES)))
    return np.concatenate([res.results[c]["alpha"] for c in range(N_CORES)],
                          axis=0)


# revision 3
# speedup vs baseline: 1.3655x; 1.3161x over previous
"""Bahdanau attention (B=32, S=2048, ENC2=1024, ATT=512) on 8 TRN2
NeuronCores, data-parallel over batch (4 batches/core), weights replicated.

v6: measured-rate rebalance. The PE on this silicon runs bf16 matmuls at
~2x the cost model (512-MM probe: 57.5us vs 109.2 modeled), so the PE
takes the enc transposes back from the DMA xbar (whose transposes cannot
overlap other DMA without a 2.3x mutual slowdown) and also does the
energy reduction directly:
  loads: plain f32 HWDGE DMA, free-running, 2 blocks ahead  (~38us)
  cast:  f32->bf16 split Pool (c0,c1) / DVE (c2,c3)
  encT:  per-[128,128] PE transposes (bf16), evac DVE/ACT
  UhT  = bf16(U)^T-chunk @ encT        [a 128, s 512] psum tiles
  tanh = tanh(UhT + Ws) via ACT bias fusion (per-partition bias)
  en   = sum_m v_m^T @ tanh_m          (4 direct PE matmuls per block:
         only two cross-engine hops ACT->PE->ACT on the energy path)
  alpha= exp(en)/sum exp(en)           (ACT exp + fused row-sum)

Output alpha [32, 2048] fp32, gathered from the 8 cores.
"""

import numpy as np

import concourse.bass as bass
import concourse.mybir as mybir
import concourse.tile as tile
from concourse import bacc
from concourse.masks import make_identity

F32 = mybir.dt.float32
F32R = mybir.dt.float32r
BF16 = mybir.dt.bfloat16

N_CORES = 8
B_FULL, S, E, A = 32, 2048, 1024, 512
B_SH = B_FULL // N_CORES          # 4 batches per core
SBLK = 512                        # s-block (matmul N)
N_SBLK = S // SBLK                # 4 per batch
EJ = E // 128                     # 8 e-chunks
AM = A // 128                     # 4 a-chunks
CC = SBLK // 128                  # 4 s-subchunks per s-block


def r(ap):
    return ap.bitcast(F32R)


def build_program(reps=1):
    nc = bacc.Bacc("TRN2", target_bir_lowering=False, debug=False,
                   num_devices=N_CORES)

    dec = nc.dram_tensor("decoder_hidden", [B_SH, E], F32R, kind="ExternalInput")
    enc = nc.dram_tensor("encoder_all_hidden", [B_SH, S, E], F32,
                         kind="ExternalInput")
    W_w = nc.dram_tensor("W_w", [E, A], F32R, kind="ExternalInput")
    W_b = nc.dram_tensor("W_b", [A], F32R, kind="ExternalInput")
    U_w = nc.dram_tensor("U_w", [E, A], F32R, kind="ExternalInput")
    U_b = nc.dram_tensor("U_b", [A], F32R, kind="ExternalInput")
    v_w = nc.dram_tensor("v_w", [A, 1], F32R, kind="ExternalInput")
    alpha = nc.dram_tensor("alpha", [B_SH, S], F32, kind="ExternalOutput")

    with tile.TileContext(nc) as tc:
        with (
            tc.tile_pool(name="const", bufs=1) as constp,
            tc.tile_pool(name="x4", bufs=3) as x4p,
            tc.tile_pool(name="x4b", bufs=2) as x4bp,
            tc.tile_pool(name="e8", bufs=3) as e8p,
            tc.tile_pool(name="tanh", bufs=9) as tanhp,
            tc.tile_pool(name="epi", bufs=2) as epip,
            tc.tile_pool(name="psT", bufs=2, space="PSUM") as psTp,
            tc.tile_pool(name="psUh", bufs=4, space="PSUM") as psUhp,
            tc.tile_pool(name="psE", bufs=2, space="PSUM") as psEp,
        ):
            # ---------------- prologue: tiny inputs ---------------------
            ident_f32 = constp.tile([128, 128], F32, tag="identf")
            make_identity(nc, ident_f32)
            ident = constp.tile([128, 128], F32R, tag="ident")
            nc.vector.tensor_copy(ident, ident_f32)
            identb = constp.tile([128, 128], BF16, tag="identb")
            nc.vector.tensor_copy(identb, ident_f32)

            dec_sb = constp.tile([B_SH, E], F32R, tag="dec")
            nc.sync.dma_start(dec_sb, dec[:, :])
            wb_sb = constp.tile([1, A], F32R, tag="wb")
            nc.sync.dma_start(wb_sb, W_b[None, :])
            ub_sb = constp.tile([1, A], F32R, tag="ub")
            nc.sync.dma_start(ub_sb, U_b[None, :])
            v_sb = constp.tile([128, AM], F32R, tag="v")
            nc.sync.dma_start(v_sb.rearrange("p (c o) -> p c o", c=AM),
                              v_w.rearrange("(c p) o -> p c o", p=128))

            # dec transposes first: PE work that's ready immediately
            dect = constp.tile([128, B_SH * EJ], F32R, tag="dect")
            for j in range(EJ):
                pst = psUhp.tile([128, SBLK], F32, tag="psUh",
                                 name=f"psTd_{j}").bitcast(F32R)
                nc.tensor.transpose(pst[:, :B_SH],
                                    r(dec_sb[:, 128 * j:128 * (j + 1)]),
                                    r(ident[:B_SH, :B_SH]))
                nc.scalar.copy(dect[:, B_SH * j:B_SH * (j + 1)], pst[:, :B_SH])

            # weights
            ww = constp.tile([128, EJ * A], F32R, tag="ww")
            nc.sync.dma_start(ww.rearrange("e (j a) -> e j a", j=EJ),
                              W_w.rearrange("(j e) a -> e j a", e=128))
            uw = constp.tile([128, EJ * A], F32R, tag="uw")
            uwb = constp.tile([128, EJ * A], BF16, tag="uwb")
            for j in range(EJ):
                sl = slice(A * j, A * (j + 1))
                nc.sync.dma_start(uw[:, sl], U_w[128 * j:128 * (j + 1), :])
                e0 = nc.gpsimd if j % 2 == 0 else nc.vector
                e0.tensor_copy(uwb[:, sl], uw.bitcast(F32)[:, sl])

            bias_sum = constp.tile([1, A], F32R, tag="bias")
            nc.vector.tensor_tensor(out=bias_sum, in0=wb_sb, in1=ub_sb,
                                    op=mybir.AluOpType.add)
            ones14f = constp.tile([1, B_SH], F32, tag="onesf")
            nc.vector.memset(ones14f, 1.0)
            ones14 = constp.tile([1, B_SH], F32R, tag="ones")
            nc.vector.tensor_copy(ones14, ones14f)

            wst = constp.tile([128, AM * B_SH], F32, tag="wst")

            def prologue_part2():
                # Ws = dec @ W_w + (W_b + U_b):  psum [B_SH, A]
                ps_ws = psEp.tile([B_SH, A], F32, tag="psE", name="ps_ws")
                for j in range(EJ):
                    nc.tensor.matmul(ps_ws,
                                     r(dect[:, B_SH * j:B_SH * (j + 1)]),
                                     r(ww[:, A * j:A * (j + 1)]),
                                     start=(j == 0), stop=False)
                nc.tensor.matmul(ps_ws, r(ones14), r(bias_sum),
                                 start=False, stop=True)
                ws_sb = constp.tile([B_SH, A], F32R, tag="ws", name="ws_sb")
                nc.scalar.copy(ws_sb, ps_ws)
                # WsT [128 a', (m b)]: col 4m+b = Ws[b, 128m + p]
                for m in range(AM):
                    pst = psUhp.tile([128, SBLK], F32, tag="psUh",
                                     name=f"pst_ws_{m}").bitcast(F32R)
                    nc.tensor.transpose(pst[:, :B_SH],
                                        r(ws_sb[:, 128 * m:128 * (m + 1)]),
                                        r(ident[:B_SH, :B_SH]))
                    nc.scalar.copy(wst[:, B_SH * m:B_SH * (m + 1)],
                                   pst[:, :B_SH])

            # ---------------- main loop ---------------------------------
            blocks = [(rep, b, sblk)
                      for rep in range(reps)
                      for b in range(B_SH)
                      for sblk in range(N_SBLK)]
            n = len(blocks)

            x4_tiles = {}
            x4b_tiles = {}
            e8_tiles = {}
            th_tiles = {}
            batch_state = {}   # (rep, b) -> (exp_b, den_b)
            pending = []       # block indices with tanh done, energy not

            def get_batch_state(rep, b):
                key = (rep, b)
                if key not in batch_state:
                    exp_b = epip.tile([1, S], F32, tag="exp",
                                      name=f"exp_{rep}_{b}")
                    den_b = epip.tile([1, N_SBLK], F32, tag="den",
                                      name=f"den_{rep}_{b}")
                    batch_state[key] = (exp_b, den_b)
                return batch_state[key]

            def issue_load(g):
                rep, b, sblk = blocks[g]
                s0 = SBLK * sblk
                x4 = x4p.tile([128, CC * E], F32, tag="x4",
                              name=f"x4_{g}")
                nc.sync.dma_start(
                    x4.rearrange("p (c e) -> p c e", c=CC),
                    enc[b, s0:s0 + SBLK, :]
                    .rearrange("(c p) e -> p c e", p=128))
                x4_tiles[g] = x4

            def issue_cast(g):
                # f32 -> bf16 split Pool / DVE
                x4 = x4_tiles.pop(g)
                x4b = x4bp.tile([128, CC * E], BF16, tag="x4b",
                                name=f"x4b_{g}")
                h = CC * E // 2
                nc.gpsimd.tensor_copy(x4b[:, :h], x4.bitcast(F32)[:, :h])
                nc.vector.tensor_copy(x4b[:, h:], x4.bitcast(F32)[:, h:])
                x4b_tiles[g] = x4b

            def transpose_evac(g, js):
                # PE transposes of [128,128] bf16 tiles; one psT tile per
                # j holds the 4 c-subchunks; evac to e8 (j s) layout
                if g not in e8_tiles:
                    e8 = e8p.tile([128, EJ * SBLK], BF16, tag="e8",
                                  name=f"e8_{g}")
                    e8_tiles[g] = e8
                else:
                    e8 = e8_tiles[g]
                x4b = x4b_tiles[g]
                for j in js:
                    pst = psTp.tile([128, SBLK], BF16, tag="psT")
                    for c in range(CC):
                        nc.tensor.transpose(
                            pst[:, 128 * c:128 * (c + 1)],
                            x4b[:, E * c + 128 * j:E * c + 128 * (j + 1)],
                            identb)
                    sl = slice(SBLK * j, SBLK * (j + 1))
                    if j in (2, 5):
                        nc.scalar.copy(e8[:, sl], pst)
                    else:
                        nc.vector.tensor_copy(e8[:, sl], pst)

            def flush_energy():
                g = pending.pop(0)
                rep, b, sblk = blocks[g]
                exp_b, den_b = get_batch_state(rep, b)
                s0 = SBLK * sblk
                ths = th_tiles.pop(g)
                # direct 4-matmul v-reduction on the (fast) PE: only two
                # cross-engine hops on the energy path
                ps_e = psEp.tile([1, SBLK], F32, tag="psE",
                                 name=f"psE_{rep}_{b}_{sblk}")
                for m in range(AM):
                    nc.tensor.matmul(ps_e, r(v_sb[:, m:m + 1]),
                                     r(ths[m]), start=(m == 0),
                                     stop=(m == AM - 1))
                nc.scalar.activation(
                    out=exp_b[:, s0:s0 + SBLK], in_=ps_e,
                    func=mybir.ActivationFunctionType.Exp,
                    accum_out=den_b[:, sblk:sblk + 1])
                if sblk == N_SBLK - 1:
                    finish_batch(rep, b, exp_b, den_b)

            def finish_batch(rep, b, exp_b, den_b):
                # softmax epilogue (no max subtraction; |energy| <= 22.6)
                dsum_b = epip.tile([1, 1], F32, tag="dsum",
                                   name=f"dsum_{rep}_{b}")
                nc.vector.reduce_sum(dsum_b, den_b,
                                     axis=mybir.AxisListType.X)
                inv_b = epip.tile([1, 1], F32, tag="inv",
                                  name=f"inv_{rep}_{b}")
                nc.vector.reciprocal(inv_b, dsum_b)
                h = S // 2
                nc.gpsimd.tensor_scalar_mul(exp_b[:, :h], exp_b[:, :h],
                                            inv_b)
                nc.vector.tensor_scalar_mul(exp_b[:, h:], exp_b[:, h:],
                                            inv_b)
                nc.sync.dma_start(alpha[b:b + 1, :h], exp_b[:, :h])
                nc.scalar.dma_start(alpha[b:b + 1, h:], exp_b[:, h:])
                del batch_state[(rep, b)]

            def mm_chain(g, m, ths):
                rep, b, sblk = blocks[g]
                e8 = e8_tiles[g]
                ps_uh = psUhp.tile([128, SBLK], F32, tag="psUh")
                for j in range(EJ):
                    nc.tensor.matmul(
                        ps_uh,
                        uwb[:, A * j + 128 * m:A * j + 128 * (m + 1)],
                        e8[:, SBLK * j:SBLK * (j + 1)],
                        start=(j == 0), stop=(j == EJ - 1))
                th = tanhp.tile([128, SBLK], F32R, tag="tanh",
                                name=f"tanh_{g}_{m}")
                nc.scalar.activation(
                    out=th, in_=ps_uh,
                    func=mybir.ActivationFunctionType.Tanh,
                    bias=wst[:, B_SH * m + b:B_SH * m + b + 1])
                ths.append(th)
                if m == AM - 1:
                    th_tiles[g] = ths
                    pending.append(g)
                    del e8_tiles[g]

            # pipeline: loads 2 ahead, cast 1 ahead; transposes of block
            # g+1 interleave between the MM chains of block g on the PE
            # queue so the cast has a chain-length window to land
            issue_load(0)
            issue_load(1)
            issue_cast(0)
            prologue_part2()
            transpose_evac(0, range(EJ))
            for g in range(n):
                if g + 2 < n:
                    issue_load(g + 2)
                if g + 1 < n:
                    issue_cast(g + 1)
                ths = []
                mm_chain(g, 0, ths)
                mm_chain(g, 1, ths)
                if g + 1 < n:
                    transpose_evac(g + 1, range(0, 4))
                mm_chain(g, 2, ths)
                mm_chain(g, 3, ths)
                if g + 1 < n:
                    transpose_evac(g + 1, range(4, EJ))
                if pending and (g >= 1 or len(pending) > 1):
                    flush_energy()
            while pending:
                flush_energy()

    nc.compile()
    return nc


def shard_inputs(inputs):
    """Full inputs dict -> list of 8 per-core input dicts."""
    dec = np.ascontiguousarray(inputs["decoder_hidden"], dtype=np.float32)
    enc = np.ascontiguousarray(inputs["encoder_all_hidden"], dtype=np.float32)
    base = {
        "W_w": np.ascontiguousarray(inputs["W_w"], dtype=np.float32),
        "W_b": np.ascontiguousarray(inputs["W_b"], dtype=np.float32),
        "U_w": np.ascontiguousarray(inputs["U_w"], dtype=np.float32),
        "U_b": np.ascontiguousarray(inputs["U_b"], dtype=np.float32),
        "v_w": np.ascontiguousarray(inputs["v_w"], dtype=np.float32),
    }
    maps = []
    for c in range(N_CORES):
        m = dict(base)
        m["decoder_hidden"] = dec[c * B_SH:(c + 1) * B_SH]
        m["encoder_all_hidden"] = enc[c * B_SH:(c + 1) * B_SH]
        maps.append(m)
    return maps


_NC_CACHE = None


def get_program():
    global _NC_CACHE
    if _NC_CACHE is None:
        _NC_CACHE = build_program()
    return _NC_CACHE


def kernel(**inputs):
    from concourse import bass_utils
    nc = get_program()
    maps = shard_inputs(inputs)
    res = bass_utils.run_bass_kernel_spmd(nc, maps,
                                          core_ids=list(range(N_CORES)))
    return np.concatenate([res.results[c]["alpha"] for c in range(N_CORES)],
                          axis=0)
